# revision 2
# baseline (speedup 1.0000x reference)
"""Trainium2 Bass kernel v2 for nn_EnhancedGNN on 8 NeuronCores.

Key structure (vs v1):
- Nodes relabeled: in-degree-balanced assignment into 8 cores x 98 tiles so
  every tile sees ~2040 in-edges (tight max-over-core chunk counts).
- Node table stored as 4 quartile-block DRAM tensors hq[b] (rows =
  [core0 q-slice | core1 q-slice | ...], <=25600 rows so gather indices fit
  int16).  Layer-to-layer AllGather is split into 4 sub-AllGathers (one per
  quartile) issued right before the bucket that needs them -> next layer's
  gathers overlap the previous layer's apply/writeback/AllGather tail.
- One gather call per (source-quartile b, 512-col dst group g): ~17 chunks,
  ~2176 indices.  Per-core valid index count is loaded into a gpsimd
  register per call; padding index slots hold -1 and are SKIPPED by the
  SWDGE ucode (no DMA descriptors, no HBM traffic).
- Scatter one-hots stay 128 wide: each chunk gets a shared data-derived
  column window [o, o+w) (w=128, rarely 256) inside the 512-col group that
  covers every core's edge span for that chunk slot.
- PSUM group accumulator [128, 512] is zero-initialized by a matmul with an
  all-zero rhs, chunk matmuls accumulate into windows, one dummy stop
  matmul closes the bank.
- Writebacks (SBUF -> hown quartile, 256B rows) rotate across the sync /
  scalar / vector engine DMA queues.
"""
import dataclasses
import os
import sys
import types
import numpy as np

NOSKIP = bool(int(os.environ.get('K2_NOSKIP', '0')))
NOPIPE = bool(int(os.environ.get('K2_NOPIPE', '0')))
NOCC = bool(int(os.environ.get('K2_NOCC', '0')))
SPLIT1024 = bool(int(os.environ.get('K2_SPLIT1024', '0')))

# ---------------------------------------------------------------------------
# harness patches (walrus on this image encodes at most ONE sync wait per
# instruction; the axon NTFF profile hook is missing from the shipped antenv)
# ---------------------------------------------------------------------------


def _apply_tile_drain_patch():
    import concourse.tile as tile_mod
    from concourse.vector_clock import ScopedClock, VectorClock

    def _patched(self, tick_clock, wait_clock):
        nc = self.nc
        gc = tick_clock.global_clock
        n = len(gc)
        for i in range(n):
            t = gc[i]
            if t <= 0:
                continue
            vec = [0] * n
            vec[i] = t
            d = nc.sync.drain()
            wait_clock.add_sem_waits(d.ins, ScopedClock({None: VectorClock(vec)}))
        nc.sync.drain()
        nc.all_engine_barrier()
        assert self.sems is not None
        popped = nc._tile_sem_poison_stack.pop()
        assert popped is self._sem_poison
        nc.clear_and_free_semaphores(list(self.sems.allocated().values()))
        nc.all_engine_barrier()

    tile_mod.TileContext._drain_and_barrier = _patched


def _split_sync_waits(nc, max_waits=1):
    import concourse.mybir as mybir
    n_split = 0
    for f in nc.m.functions:
        for blk in f.blocks:
            new_insts = []
            for ins in blk.instructions:
                si = ins.sync_info
                if si is not None and si.on_wait and len(si.on_wait) > max_waits:
                    waits = list(si.on_wait)
                    keep = waits[-max_waits:]
                    for w in waits[:-max_waits]:
                        nop = mybir.InstNoOp(
                            name=f"{ins.name}-ws{n_split}", ins=[], outs=[])
                        nop.engine = ins.engine
                        nop.sync_info = mybir.SyncInfo(on_wait=[w], on_update=[])
                        new_insts.append(nop)
                        n_split += 1
                    si.on_wait = keep
                new_insts.append(ins)
            blk.instructions[:] = new_insts
    return n_split


def _install_ntff_hook():
    if 'antenv.axon_hooks' in sys.modules:
        return
    try:
        from trn_agent_boot.trn_boot import _ntff_profile_via_ctypes
        hook = _ntff_profile_via_ctypes('/opt/axon/libaxon_pjrt.so')
    except Exception:
        hook = None
    mod = types.ModuleType('antenv.axon_hooks')
    state = {'hook': hook}
    mod.get_axon_ntff_profile_hook = lambda: state['hook']
    mod.set_axon_ntff_profile_hook = lambda h: state.update(hook=h)
    sys.modules['antenv.axon_hooks'] = mod
    try:
        import antenv
        antenv.axon_hooks = mod
    except Exception:
        pass
    try:
        import concourse.bass_utils as bu
        bu.upload_artifacts = lambda tmpdir: tmpdir
    except Exception:
        pass


def _maybe_reset_device():
    try:
        import jax, ctypes
        jax.devices()
        lib = ctypes.CDLL('/opt/axon/libaxon_pjrt.so')
        lib.axon_reset.restype = ctypes.c_int64
        lib.axon_reset()
    except Exception:
        pass


# ---------------------------------------------------------------------------
# constants
# ---------------------------------------------------------------------------

H = 128
DOUT = 40
BN_EPS = 1e-5
NCORES = 8
P = 128
OWN = 12500
OWNP = 12544          # 98 tiles
T = 98
G = 25                # 512-col dst groups (last is 256 wide)
NB = 4                # source-quartile buckets
QS = [3200, 3200, 3200, 2944]       # quartile sizes (local rows)
QSTART = [0, 3200, 6400, 9600]
QT = [25, 25, 25, 23]               # tiles per quartile
QT0 = [0, 25, 50, 75, 98]           # tile ranges per quartile
GW = [512] * 24 + [256]
MIN_CNT = 16
GBUFS = 8      # gather/onehot pool buffers (in-flight calls)
AHEAD = 4      # call prefetch depth beyond current group's needs
IDXP = 16 if bool(int(os.environ.get('K2_IDX16', '0'))) else 128


@dataclasses.dataclass
class Plan2:
    ncalls: int
    gcalls: list    # per bucket: list of call dicts (<=8 slots each)
    groups: list    # per bucket: per g: [(call_k, pos, o, w, fam, fpos)]
    c128: int
    c256: int
    max_stw: int                    # max one-hot cols per call
    idx_len16: int                  # total idx stream length (16-lane units)
    bucket_idx16: list              # per bucket (start16, len16)
    idx_w: list                     # per core [128, idx_len16] int16
    ds128: list                     # per core [128, c128] f16
    ds256: list                     # per core [128, c256] f16
    cnt: list                       # per core [128, ncalls] int32
    invdeg: list                    # per core [128, OWNP] f16
    xT: list                        # per core [128, OWNP] f16
    xq: list                        # per bucket [8*QS[b], H] f16 (shared)
    orig_ids: list                  # per core [OWN] original node ids


def build_plan2(x, edge_index):
    N = x.shape[0]
    src = edge_index[0].astype(np.int64)
    dst = edge_index[1].astype(np.int64)

    # ---- degree-balanced node -> (core, local slot) assignment ----------
    deg = np.bincount(dst, minlength=N).astype(np.int64)
    order = np.argsort(-deg, kind='stable')
    nbins = NCORES * T
    cap = np.full(nbins, P, np.int64)
    cap[T - 1::T] = 84            # tile 97 of each core: 84 real + 44 pad
    import heapq
    heap = [(0, b) for b in range(nbins)]
    heapq.heapify(heap)
    fill = np.zeros(nbins, np.int64)
    core_of = np.empty(N, np.int32)
    loc_of = np.empty(N, np.int32)
    # bins assign slots in arrival order
    for v in order:
        while True:
            load, b = heapq.heappop(heap)
            if fill[b] < cap[b]:
                break
        c, t = divmod(b, T)
        core_of[v] = c
        loc_of[v] = t * P + fill[b]
        fill[b] += 1
        if fill[b] < cap[b]:
            heapq.heappush(heap, (load + deg[v], b))
    assert (fill == cap).all()

    orig_ids = []
    for c in range(NCORES):
        ids = np.full(OWN, -1, np.int64)
        m = core_of == c
        l = loc_of[m]
        # compress local slots (tile 97 has pads beyond 84 -> local ids of
        # real nodes are 0..12499 after shifting? no: slots are dense within
        # tiles; tile 97 fills 0..83 -> locals 12416..12499. dense already.)
        ids[l - (l // P - (T - 1)).clip(0) * 0] = np.where(m)[0]  # direct
        # simpler: direct placement
        ids = np.full(OWNP, -1, np.int64)
        ids[l] = np.where(m)[0]
        real = ids[ids >= 0]
        assert real.size == OWN
        # real locals are exactly 0..12499 by construction
        assert (ids[:OWN] >= 0).all() and (ids[OWN:] < 0).all()
        orig_ids.append(ids[:OWN])

    # ---- remap edges ----------------------------------------------------
    sc = core_of[src].astype(np.int64)
    sl = loc_of[src].astype(np.int64)
    dc = core_of[dst].astype(np.int64)
    dl = loc_of[dst].astype(np.int64)
    b_of = np.minimum(sl // 3200, 3)
    qs_arr = np.array(QS, np.int64)
    qstart_arr = np.array(QSTART, np.int64)
    idx16_of = sc * qs_arr[b_of] + (sl - qstart_arr[b_of])
    assert idx16_of.max() < 32768
    g_of = dl >> 9
    col_of = dl & 511

    # per-core sorted edge segments
    percore = []
    for c in range(NCORES):
        m = dc == c
        bb, gg, cc2, ii = b_of[m], g_of[m], col_of[m], idx16_of[m]
        o = np.lexsort((cc2, gg, bb))
        percore.append((bb[o], gg[o], cc2[o], ii[o]))

    # segment boundaries per (b, g) per core
    seg = np.zeros((NCORES, NB, G + 1), np.int64)
    for c in range(NCORES):
        bb, gg, _, _ = percore[c]
        key = bb * G + gg
        cnts = np.bincount(key, minlength=NB * G).reshape(NB, G)
        seg[c, :, 1:] = np.cumsum(cnts, axis=1)
        seg[c] += np.concatenate(
            [[0], np.cumsum(cnts.sum(axis=1))])[:-1, None]

    # ---- slot construction, then cross-group 8-slot call packing --------
    gcalls = [[] for _ in range(NB)]     # per bucket: list of call dicts
    groups = [[] for _ in range(NB)]     # per bucket: per g: slot refs
    idx_blocks = [[] for _ in range(NCORES)]
    ds128_cols = [[] for _ in range(NCORES)]
    ds256_cols = [[] for _ in range(NCORES)]
    cnt_rows = [[] for _ in range(NCORES)]
    bucket_idx16 = []
    idx_pos16 = 0
    max_stw = 0
    for b in range(NB):
        bstart16 = idx_pos16
        slot_recs = []                   # dicts: g, o, w, blocks, dscols, pad
        for g in range(G):
            gw = GW[g]
            e = [int(seg[c, b, g + 1] - seg[c, b, g]) for c in range(NCORES)]
            nsl = max(1, -(-max(e) // P))
            for j in range(nsl):
                lo, hi = 10 ** 9, -1
                for c in range(NCORES):
                    if e[c] <= j * P:
                        continue
                    s0 = int(seg[c, b, g])
                    cols = percore[c][2][s0 + j * P:s0 + min((j + 1) * P,
                                                             e[c])]
                    lo = min(lo, int(cols[0]))
                    hi = max(hi, int(cols[-1]))
                if hi < 0:
                    lo, hi = 0, 0
                span = hi - lo + 1
                w = 128 if span <= 128 else 256
                assert span <= 256, f"window span {span} at b{b} g{g} j{j}"
                w = min(w, gw)
                o = min(lo, gw - w)
                blocks, dscols, pad = [], [], 0
                for c in range(NCORES):
                    s0 = int(seg[c, b, g])
                    n = min(max(e[c] - j * P, 0), P)
                    blk = np.full(P, -1, np.int16)
                    dsv = np.full(P, -1.0, np.float32)
                    if n > 0:
                        blk[:n] = percore[c][3][s0 + j * P:
                                                s0 + j * P + n].astype(
                                                    np.int16)
                        dsv[:n] = percore[c][2][s0 + j * P:s0 + j * P + n] - o
                    blocks.append(blk)
                    dscols.append(dsv)
                    pad += P - n
                slot_recs.append(dict(g=g, o=o, w=w, blocks=blocks,
                                      dscols=dscols, pad=pad))
        # pack into calls of <=8 slots; within each window put the slot
        # with the most padding last so its pads become skippable
        for w0 in range(0, len(slot_recs), 8):
            win = slot_recs[w0:w0 + 8]
            last = max(range(len(win)), key=lambda i: win[i]['pad'])
            win = [win[i] for i in range(len(win)) if i != last] + [win[last]]
            nsl = len(win)
            nidx = nsl * P
            kloc = len(gcalls[b])
            n128 = sum(1 for s in win if s['w'] == 128)
            n256 = nsl - n128
            max_stw = max(max_stw, n128 * 128 + n256 * 256)
            # per-core idx assembly + counts
            for c in range(NCORES):
                lanes = []
                for pos, s in enumerate(win):
                    lane = s['blocks'][c].copy()
                    if pos < nsl - 1:
                        lane[lane < 0] = 0       # interior pads -> dummies
                    lanes.append(lane)
                blk = np.concatenate(lanes)
                evalid = int(np.max(np.nonzero(blk >= 0)[0],
                                    initial=-1)) + 1
                cnt_c = min(max(MIN_CNT, -(-evalid // 16) * 16), nidx)
                if NOSKIP:
                    cnt_c = nidx
                blk[evalid:cnt_c] = 0
                idx_blocks[c].append(blk)
                cnt_rows[c].append(cnt_c)
            # ds stream in call-slot order + group slot refs
            i128 = i256 = 0
            for pos, s in enumerate(win):
                if s['w'] == 128:
                    fam, fpos = 0, i128
                    i128 += 1
                    for c in range(NCORES):
                        ds128_cols[c].append(s['dscols'][c])
                else:
                    fam, fpos = 1, i256
                    i256 += 1
                    for c in range(NCORES):
                        ds256_cols[c].append(s['dscols'][c])
                while len(groups[b]) <= s['g']:
                    groups[b].append([])
                groups[b][s['g']].append((kloc, pos, s['o'], s['w'],
                                          fam, fpos))
            gcalls[b].append(dict(
                b=b, nslots=nsl, num_idxs=nidx, idx_off16=idx_pos16,
                cnt_idx=len(cnt_rows[0]) - 1, n128=n128, n256=n256,
                ds128_off=len(ds128_cols[0]) - n128,
                ds256_off=len(ds256_cols[0]) - n256))
            idx_pos16 += nidx // 16
        while len(groups[b]) < G:
            groups[b].append([])
        bucket_idx16.append((bstart16, idx_pos16 - bstart16))

    ncalls = sum(len(gc) for gc in gcalls)
    c128 = len(ds128_cols[0])
    c256 = max(1, len(ds256_cols[0]))

    idx_w, ds128_w, ds256_w, cnt_w = [], [], [], []
    for c in range(NCORES):
        flat = np.concatenate(idx_blocks[c])
        w16 = flat.reshape(-1, 16).T        # [16, idx_len16]
        idx_w.append(np.tile(w16, (8, 1)).astype(np.int16))
        d1 = np.stack(ds128_cols[c], axis=1).astype(np.float16)
        ds128_w.append(np.ascontiguousarray(d1))
        if ds256_cols[c]:
            d2 = np.stack(ds256_cols[c], axis=1).astype(np.float16)
        else:
            d2 = np.full((P, 1), -1.0, np.float16)
        ds256_w.append(np.ascontiguousarray(d2))
        cnt_w.append(np.tile(np.array(cnt_rows[c], np.int32)[None, :],
                             (P, 1)))

    # ---- invdeg / x staging --------------------------------------------
    inv_all = (1.0 / np.maximum(deg, 1.0)).astype(np.float32)
    invs, xTs = [], []
    xf = np.asarray(x, np.float32)
    for c in range(NCORES):
        iv = np.zeros(OWNP, np.float32)
        iv[:OWN] = inv_all[orig_ids[c]]
        invs.append(np.tile(iv[None, :], (P, 1)).astype(np.float16))
        xo = np.zeros((OWNP, H), np.float32)
        xo[:OWN] = xf[orig_ids[c]]
        xTs.append(np.ascontiguousarray(xo.T).astype(np.float16))

    xq = []
    for b in range(NB):
        tb = np.zeros((NCORES * QS[b], H), np.float16)
        for c in range(NCORES):
            q0, q1 = QSTART[b], QSTART[b] + QS[b]
            rows = np.zeros((QS[b], H), np.float32)
            hi = min(q1, OWN)
            if hi > q0:
                rows[:hi - q0] = xf[orig_ids[c][q0:hi]]
            tb[c * QS[b]:(c + 1) * QS[b]] = rows.astype(np.float16)
        xq.append(tb)

    return Plan2(ncalls=ncalls, gcalls=gcalls, groups=groups,
                 c128=c128, c256=c256,
                 max_stw=max_stw, idx_len16=idx_pos16,
                 bucket_idx16=bucket_idx16, idx_w=idx_w, ds128=ds128_w,
                 ds256=ds256_w, cnt=cnt_w, invdeg=invs, xT=xTs, xq=xq,
                 orig_ids=orig_ids)


def _bcast_mid(ap, reps):
    return dataclasses.replace(ap, ap=[ap.ap[0], [0, reps], ap.ap[1]])


def _bcast_last(ap, reps):
    return dataclasses.replace(ap, ap=[ap.ap[0], ap.ap[1], [0, reps]])


def build_bass2(pl):
    import concourse.bass as bass
    import concourse.mybir as mybir
    from concourse.tile import TileContext
    from concourse import library_config
    from concourse.library_overlay import lower_extended_insts

    F16, F32, I16, I32 = (mybir.dt.float16, mybir.dt.float32,
                          mybir.dt.int16, mybir.dt.int32)
    AX = mybir.AxisListType
    ALU = mybir.AluOpType
    ACTF = mybir.ActivationFunctionType

    nc = bass.Bass('TRN2', target_bir_lowering=False, debug=False,
                   num_devices=NCORES, num_swdge_queues=4)

    def din(name, shape, dt):
        return nc.dram_tensor(name, shape, dt, kind='ExternalInput')

    xq_d = [din(f'xq{b}', [NCORES * QS[b], H], F16) for b in range(NB)]
    xT_d = din('xT', [P, OWNP], F16)
    idx_d = din('idxw', [P, pl.idx_len16], I16)
    ds128_d = din('ds128', [P, pl.c128], F16)
    ds256_d = din('ds256', [P, pl.c256], F16)
    cnt_d = din('cnt', [P, pl.ncalls], I32)
    inv_d = din('invdeg', [P, OWNP], F16)
    iota_d = din('iota', [P, 256], F16)
    ident_d = din('ident', [P, P], F16)
    wname = ['w0l', 'w0r', 'w1', 'w2', 'w2l', 'w2r']
    w_d = {k: din(k, [H, H], F16) for k in wname}
    fwl_d = din('fwl', [H, DOUT], F16)
    fwr_d = din('fwr', [H, DOUT], F16)
    b1_d = din('b1T', [P, 1], F32)
    finb_d = din('finb', [P, DOUT], F32)
    bng_d = din('bngT', [P, 3], F32)
    bnb_d = din('bnbT', [P, 3], F32)
    OUTP = 64
    out_d = nc.dram_tensor('out', [OWNP, OUTP], F32, kind='ExternalOutput')

    hq_own_d = [nc.dram_tensor(f'hqo{b}', [QS[b], H], F16, kind='Internal')
                for b in range(NB)]
    hq_shared = not bool(int(os.environ.get('K2_NOSHARED', '0')))
    hq_d = [nc.dram_tensor(f'hq{b}', [NCORES * QS[b], H], F16,
                           kind='Internal',
                           addr_space='Shared' if hq_shared else 'Local')
            for b in range(NB)]
    stin_d = [nc.dram_tensor(f'stin{i}', [P, 2], F32, kind='Internal')
              for i in range(3)]
    stout_d = [nc.dram_tensor(f'stout{i}', [P, 2], F32, kind='Internal',
                              addr_space='Shared') for i in range(3)]
    rg = [list(range(NCORES))]

    nc.gpsimd.load_library(library_config.mlp)

    NGRP = G
    grp_w = [GW[g] for g in range(G)]
    grp_v = [min(512, max(0, OWN - g * 512)) for g in range(G)]

    with TileContext(nc) as tc:
        cnt_reg = nc.alloc_register(mybir.EngineType.Pool, 'gcnt')
        nidx_regs = {n: nc.gpsimd.to_reg(n)
                     for n in sorted({cl['num_idxs'] for gc in pl.gcalls
                                      for cl in gc})}
        import contextlib
        ctx = contextlib.ExitStack()
        with ctx:
            persist = ctx.enter_context(tc.tile_pool(name='persist', bufs=1))
            gpool = ctx.enter_context(tc.tile_pool(name='g', bufs=GBUFS))
            spool = ctx.enter_context(tc.tile_pool(name='s', bufs=GBUFS))
            ipool = ctx.enter_context(tc.tile_pool(name='idx', bufs=2))
            epool = ctx.enter_context(tc.tile_pool(name='evac', bufs=3))
            psA = ctx.enter_context(
                tc.tile_pool(name='psA', bufs=3, space='PSUM'))
            psD = ctx.enter_context(
                tc.tile_pool(name='psD', bufs=2, space='PSUM'))
            psT = ctx.enter_context(
                tc.tile_pool(name='psT', bufs=2, space='PSUM'))

            def load(name, shape, dt, srcap):
                t = persist.tile(shape, dt, tag=name)
                nc.sync.dma_start(t[:], srcap)
                return t

            ds128_sb = load('ds128', [P, pl.c128], F16, ds128_d[:])
            ds256_sb = load('ds256', [P, pl.c256], F16, ds256_d[:])
            cnt_sb = load('cnt', [P, pl.ncalls], I32, cnt_d[:])
            inv_sb = load('inv', [P, OWNP], F16, inv_d[:])
            iota_sb = load('iota', [P, 256], F16, iota_d[:])
            ident_sb = load('ident', [P, P], F16, ident_d[:])
            w_sb = {k: load(k, [H, H], F16, w_d[k][:]) for k in wname}
            fwl_sb = load('fwl', [H, DOUT], F16, fwl_d[:])
            fwr_sb = load('fwr', [H, DOUT], F16, fwr_d[:])
            b1_sb = load('b1', [P, 1], F32, b1_d[:])
            finb_sb = load('finb', [P, DOUT], F32, finb_d[:])
            bng_sb = load('bng', [P, 3], F32, bng_d[:])
            bnb_sb = load('bnb', [P, 3], F32, bnb_d[:])
            xqsb = load('xq', [P, OWNP], F16, xT_d[:])

            hA = persist.tile([P, OWNP], F16, tag='hA')
            hB = persist.tile([P, OWNP], F16, tag='hB')
            aggT = persist.tile([P, OWNP], F16, tag='aggT')
            zero512 = persist.tile([P, 512], F16, tag='zero512')
            sums = persist.tile([P, NGRP], F32, tag='sums')
            sqs = persist.tile([P, NGRP], F32, tag='sqs')
            stat = persist.tile([P, 2], F32, tag='stat')
            gstat = persist.tile([P, 2], F32, tag='gstat')
            scl = persist.tile([P, 1], F32, tag='scl')
            bia = persist.tile([P, 1], F32, tag='bia')
            tmp1 = persist.tile([P, 1], F32, tag='tmp1')
            tmp2 = persist.tile([P, 1], F32, tag='tmp2')
            ttscr = persist.tile([P, 512], F32, tag='ttscr')
            logit = persist.tile([P, T * DOUT], F32, tag='logit')
            mx = persist.tile([P, T], F32, tag='mx')
            lse = persist.tile([P, T], F32, tag='lse')
            escr = persist.tile([P, DOUT], F32, tag='escr')

            nc.vector.memset(zero512[:], 0.0)

            # zero the gather-pool buffers once (pad slots are skipped by
            # SWDGE and must never expose NaN bit patterns to the matmul)
            maxsl = 8
            for _ in range(GBUFS):
                t = gpool.tile([P, maxsl, P], F16, tag='g')
                nc.vector.memset(
                    t[:].rearrange('p a b -> p (a b)'), 0.0)

            # collective warm-up — emitted a few gather calls into layer 0
            # so the first gathers start immediately at kernel entry
            def warm_cb():
                for i in range(3):
                    nc.gpsimd.collective_compute(
                        'AllReduce', ALU.add, rg, ins=[stin_d[i][:]],
                        outs=[stout_d[i][:]])
                if not NOCC:
                    for b in range(NB):
                        nc.gpsimd.collective_compute(
                            'AllGather', ALU.bypass, rg,
                            ins=[hq_own_d[b][0:2, :]],
                            outs=[hq_d[b][0:2 * NCORES, :]])

            # ---------------- aggregation -----------------------------
            def aggregate(tables, gin, cur_h, group_cb, ag_issue=None,
                          mid_cb=None):
                for b in range(NB):
                    if ag_issue is not None:
                        ag_issue[b]()
                    s16, l16 = pl.bucket_idx16[b]
                    h1 = (l16 // 2 + 7) // 8 * 8
                    it = ipool.tile([P, l16], I16, tag='idx')
                    nc.scalar.dma_start(it[:, :h1], idx_d[:, s16:s16 + h1])
                    nc.scalar.dma_start(it[:, h1:l16],
                                        idx_d[:, s16 + h1:s16 + l16])
                    bcalls = pl.gcalls[b]
                    # last group consuming each call (for pend release)
                    last_grp = {}
                    for g in range(G):
                        for (kk, *_rest) in pl.groups[b][g]:
                            last_grp[kk] = g
                    pend = {}
                    emitted = [0]

                    def emit_upto(klim, itile, s16=s16, bcalls=bcalls,
                                  pend=pend, emitted=emitted):
                        while emitted[0] < min(klim, len(bcalls)):
                            k = emitted[0]
                            cl = bcalls[k]
                            nsl = cl['nslots']
                            sni = cl['num_idxs']
                            if NOSKIP:
                                nreg = nidx_regs[sni]
                            else:
                                nc.gpsimd.reg_load(
                                    cnt_reg,
                                    cnt_sb[0:1, cl['cnt_idx']:
                                           cl['cnt_idx'] + 1])
                                nreg = cnt_reg
                            gt = gpool.tile([P, maxsl, P], F16, tag='g')
                            nc.gpsimd.dma_gather(
                                gt[:, :nsl, :], tables[b][:],
                                itile[0:IDXP,
                                      cl['idx_off16'] - s16:
                                      cl['idx_off16'] - s16 + sni // 16],
                                num_idxs=sni, num_idxs_reg=nreg,
                                elem_size=H, queue_num=k % 4)
                            n128, n256 = cl['n128'], cl['n256']
                            st = spool.tile([P, pl.max_stw], F16, tag='s')
                            if n128:
                                o = cl['ds128_off']
                                nc.vector.tensor_tensor(
                                    out=st[:, :n128 * 128].rearrange(
                                        'p (c f) -> p c f', c=n128),
                                    in0=_bcast_mid(iota_sb[:, :128], n128),
                                    in1=_bcast_last(
                                        ds128_sb[:, o:o + n128], 128),
                                    op=ALU.is_equal)
                            if n256:
                                o = cl['ds256_off']
                                nc.vector.tensor_tensor(
                                    out=st[:, n128 * 128:
                                           n128 * 128 + n256 * 256]
                                    .rearrange('p (c f) -> p c f', c=n256),
                                    in0=_bcast_mid(iota_sb[:, :256], n256),
                                    in1=_bcast_last(
                                        ds256_sb[:, o:o + n256], 256),
                                    op=ALU.is_equal)
                            pend[k] = (gt, st, n128)
                            emitted[0] += 1

                    for g in range(G):
                        if mid_cb is not None and b == 0 and g == 2:
                            mid_cb()
                        gw = GW[g]
                        slots = pl.groups[b][g]
                        kmax = max((kk for (kk, *_r) in slots), default=-1)
                        emit_upto(kmax + 1 + AHEAD, it)
                        pt = psA.tile([P, 512], F32, tag='agg')
                        nc.tensor.matmul(
                            pt[:, :gw], lhsT=ident_sb[:],
                            rhs=zero512[:, :gw], start=True, stop=False,
                            skip_group_check=True)
                        if gin and b == 0:
                            for tt in range(gw // 128):
                                t0 = (g * 4 + tt) * P
                                nc.tensor.matmul(
                                    pt[:, tt * 128:tt * 128 + 128],
                                    lhsT=ident_sb[:],
                                    rhs=cur_h[:, t0:t0 + P],
                                    start=False, stop=False,
                                    skip_group_check=True)
                        for (kk, pos, o, w, fam, fpos) in slots:
                            gt, st, n128 = pend[kk]
                            if fam == 0:
                                rhs = st[:, fpos * 128:(fpos + 1) * 128]
                            else:
                                rhs = st[:, n128 * 128 + fpos * 256:
                                         n128 * 128 + (fpos + 1) * 256]
                            nc.tensor.matmul(
                                pt[:, o:o + w], lhsT=gt[:, pos, :], rhs=rhs,
                                start=False, stop=False,
                                skip_group_check=True)
                        nc.tensor.matmul(
                            pt[:, :gw], lhsT=ident_sb[:],
                            rhs=zero512[:, :gw], start=False, stop=True,
                            skip_group_check=True)
                        for kk in [kk for kk, lg in last_grp.items()
                                   if lg == g]:
                            pend.pop(kk, None)
                        sl = aggT[:, g * 512:g * 512 + gw]
                        if b == 0:
                            nc.vector.tensor_copy(out=sl, in_=pt[:, :gw])
                        else:
                            nc.vector.tensor_tensor(
                                out=sl, in0=sl, in1=pt[:, :gw], op=ALU.add)
                        if b == NB - 1:
                            if not gin:
                                nc.vector.tensor_tensor(
                                    out=sl, in0=sl,
                                    in1=inv_sb[:, g * 512:g * 512 + gw],
                                    op=ALU.mult)
                            group_cb(g)

            # ---------------- dense / stats ---------------------------
            def dense_prebn(layer, g, cur):
                w = grp_w[g]
                pt = psD.tile([P, 512], F32, tag='dense')
                sl = slice(g * 512, g * 512 + w)
                if layer == 0:
                    nc.tensor.matmul(pt[:, :w], lhsT=w_sb['w0l'][:],
                                     rhs=aggT[:, sl], start=True, stop=False,
                                     skip_group_check=True)
                    nc.tensor.matmul(pt[:, :w], lhsT=w_sb['w0r'][:],
                                     rhs=xqsb[:, sl], start=False, stop=True,
                                     skip_group_check=True)
                elif layer == 1:
                    nc.tensor.matmul(pt[:, :w], lhsT=w_sb['w2'][:],
                                     rhs=xqsb[:, sl], start=True, stop=True,
                                     skip_group_check=True)
                else:
                    nc.tensor.matmul(pt[:, :w], lhsT=w_sb['w2l'][:],
                                     rhs=aggT[:, sl], start=True, stop=False,
                                     skip_group_check=True)
                    nc.tensor.matmul(pt[:, :w], lhsT=w_sb['w2r'][:],
                                     rhs=cur[:, sl], start=False, stop=True,
                                     skip_group_check=True)
                return pt

            def stats_pass1(layer, cur, scratch):
                def cb(g):
                    w, v = grp_w[g], grp_v[g]
                    pt = dense_prebn(layer, g, cur)
                    sl = slice(g * 512, g * 512 + w)
                    nc.scalar.copy(scratch[:, sl], pt[:, :w])
                    if v > 0:
                        nc.vector.tensor_reduce(
                            out=sums[:, g:g + 1], in_=pt[:, :v], axis=AX.X,
                            op=ALU.add)
                        nc.scalar.activation(
                            ttscr[:, :v], pt[:, :v], ACTF.Square,
                            accum_out=sqs[:, g:g + 1])
                    else:
                        nc.vector.memset(sums[:, g:g + 1], 0.0)
                        nc.vector.memset(sqs[:, g:g + 1], 0.0)
                return cb

            WBT = 4
            wb_engines = [nc.sync, nc.scalar]
            wb_rr = [0]

            def bn_stats_and_apply(layer, cur, nxt, residual, scratch):
                nc.vector.tensor_reduce(out=stat[:, 0:1], in_=sums[:],
                                        axis=AX.X, op=ALU.add)
                nc.vector.tensor_reduce(out=stat[:, 1:2], in_=sqs[:],
                                        axis=AX.X, op=ALU.add)
                nc.sync.dma_start(stin_d[layer][:], stat[:])
                nc.gpsimd.collective_compute(
                    'AllReduce', ALU.add, rg, ins=[stin_d[layer][:]],
                    outs=[stout_d[layer][:]])
                nc.sync.dma_start(gstat[:], stout_d[layer][:])
                invN = 1.0 / 100000.0
                nc.vector.tensor_scalar_mul(tmp1[:], gstat[:, 0:1], invN)
                nc.vector.tensor_scalar_mul(tmp2[:], gstat[:, 1:2], invN)
                nc.vector.tensor_tensor(out=scl[:], in0=tmp1[:], in1=tmp1[:],
                                        op=ALU.mult)
                nc.vector.tensor_tensor(out=tmp2[:], in0=tmp2[:], in1=scl[:],
                                        op=ALU.subtract)
                nc.vector.tensor_scalar_add(tmp2[:], tmp2[:], BN_EPS)
                nc.scalar.sqrt(tmp2[:], tmp2[:])
                nc.vector.reciprocal(tmp2[:], tmp2[:])
                nc.vector.tensor_tensor(out=scl[:],
                                        in0=bng_sb[:, layer:layer + 1],
                                        in1=tmp2[:], op=ALU.mult)
                nc.vector.tensor_tensor(out=tmp1[:], in0=tmp1[:], in1=scl[:],
                                        op=ALU.mult)
                nc.vector.tensor_tensor(out=bia[:],
                                        in0=bnb_sb[:, layer:layer + 1],
                                        in1=tmp1[:], op=ALU.subtract)

                def wb_tiles(q, t0, nt):
                    pt2 = psT.tile([P, WBT * P], F16, tag='trf16')
                    for j in range(nt):
                        t = t0 + j
                        nc.tensor.transpose(
                            pt2[:, j * P:(j + 1) * P],
                            nxt[:, t * P:(t + 1) * P], ident_sb[:])
                    et = epool.tile([P, WBT * P], F16, tag='ev')
                    nc.vector.tensor_copy(out=et[:, :nt * P],
                                          in_=pt2[:, :nt * P])
                    r0 = (t0 - QT0[q]) * P
                    dst = hq_own_d[q][r0:r0 + nt * P, :]
                    dst = dataclasses.replace(
                        dst, ap=[[H, P], [P * H, nt], [1, H]])
                    eng = wb_engines[wb_rr[0] % len(wb_engines)]
                    wb_rr[0] += 1
                    eng.dma_start(
                        dst, et[:, :nt * P].rearrange(
                            'p (j f) -> p j f', j=nt))

                # apply groups in order; writeback + AG per quartile
                qnext = [0]
                t_done = [0]

                def flush_quartile_upto(tile_lim):
                    while qnext[0] < NB and QT0[qnext[0] + 1] <= tile_lim:
                        q = qnext[0]
                        while t_done[0] < QT0[q + 1]:
                            nt = min(WBT, QT0[q + 1] - t_done[0])
                            wb_tiles(q, t_done[0], nt)
                            t_done[0] += nt
                        qnext[0] += 1

                for g in range(NGRP):
                    w = grp_w[g]
                    sl = slice(g * 512, g * 512 + w)
                    nc.scalar.activation(nxt[:, sl], scratch[:, sl],
                                         ACTF.Relu, bias=bia[:], scale=scl[:])
                    if residual:
                        nc.vector.tensor_tensor(out=nxt[:, sl],
                                                in0=nxt[:, sl],
                                                in1=cur[:, sl], op=ALU.add)
                    if g == NGRP - 1:
                        nc.vector.memset(nxt[:, OWN:OWNP], 0.0)
                        flush_quartile_upto(T)
                    else:
                        flush_quartile_upto(((g + 1) * 512) // P)

                def make_issue(q):
                    def issue():
                        if NOCC:
                            nc.sync.dma_start(hq_d[q][0:QS[q], :],
                                              hq_own_d[q][:])
                            return
                        nc.gpsimd.collective_compute(
                            'AllGather', ALU.bypass, rg,
                            ins=[hq_own_d[q][:]], outs=[hq_d[q][:]])
                    return issue
                if NOPIPE:
                    for q in range(NB):
                        make_issue(q)()
                    return [(lambda: None) for _ in range(NB)]
                return [make_issue(q) for q in range(NB)]

            # ---------------- final layer softmax ---------------------
            SM_G = [5, 11, 17, NGRP - 1]
            sm_done = [0]

            def softmax_flush(te):
                ts = sm_done[0]
                nt = te - ts
                if nt <= 0:
                    return
                lv = logit[:, ts * DOUT:te * DOUT].rearrange(
                    'p (t c) -> p t c', t=nt)
                nc.vector.tensor_reduce(out=mx[:, ts:te], in_=lv, axis=AX.X,
                                        op=ALU.max)
                nc.vector.tensor_tensor(
                    out=lv, in0=lv, in1=_bcast_last(mx[:, ts:te], DOUT),
                    op=ALU.subtract)
                for t in range(ts, te):
                    nc.scalar.activation(
                        escr[:], logit[:, t * DOUT:(t + 1) * DOUT], ACTF.Exp,
                        accum_out=lse[:, t:t + 1])
                nc.scalar.activation(lse[:, ts:te], lse[:, ts:te], ACTF.Ln)
                nc.vector.tensor_tensor(
                    out=lv, in0=lv, in1=_bcast_last(lse[:, ts:te], DOUT),
                    op=ALU.subtract)
                dstap = out_d[ts * P:te * P, :DOUT]
                dstap = dataclasses.replace(
                    dstap, ap=[[OUTP, P], [P * OUTP, nt], [1, DOUT]])
                nc.sync.dma_start(dstap, lv)
                sm_done[0] = te

            def fin_cb(g):
                n_t = min(g * 4 + 4, T) - g * 4
                for k in range(n_t):
                    t = g * 4 + k
                    pt = psD.tile([P, DOUT], F32, tag='dense')
                    nc.tensor.matmul(pt[:, :DOUT],
                                     lhsT=aggT[:, t * P:(t + 1) * P],
                                     rhs=fwl_sb[:], start=True, stop=False,
                                     skip_group_check=True)
                    nc.tensor.matmul(pt[:, :DOUT],
                                     lhsT=hA[:, t * P:(t + 1) * P],
                                     rhs=fwr_sb[:], start=False, stop=True,
                                     skip_group_check=True)
                    nc.vector.tensor_tensor(
                        out=logit[:, t * DOUT:(t + 1) * DOUT],
                        in0=pt[:, :DOUT], in1=finb_sb[:], op=ALU.add)
                if g in SM_G:
                    softmax_flush(min(g * 4 + 4, T))

            # ---------------- layer sequence --------------------------
            warm_cb()
            aggregate(xq_d, gin=False, cur_h=None,
                      group_cb=stats_pass1(0, None, hB))
            agi = bn_stats_and_apply(0, cur=None, nxt=hA, residual=False,
                                     scratch=hB)
            _p1_gin = stats_pass1(1, hA, aggT)

            def gin_cb(g):
                w = grp_w[g]
                pt = psD.tile([P, 512], F32, tag='dense')
                sl = slice(g * 512, g * 512 + w)
                nc.tensor.matmul(pt[:, :w], lhsT=w_sb['w1'][:],
                                 rhs=aggT[:, sl], start=True, stop=True,
                                 skip_group_check=True)
                nc.scalar.activation(xqsb[:, sl], pt[:, :w], ACTF.Relu,
                                     bias=b1_sb[:], scale=1.0)
                _p1_gin(g)

            aggregate(hq_d, gin=True, cur_h=hA, group_cb=gin_cb,
                      ag_issue=agi)
            agi = bn_stats_and_apply(1, cur=hA, nxt=hB, residual=True,
                                     scratch=aggT)
            aggregate(hq_d, gin=False, cur_h=None,
                      group_cb=stats_pass1(2, hB, xqsb), ag_issue=agi)
            agi = bn_stats_and_apply(2, cur=hB, nxt=hA, residual=True,
                                     scratch=xqsb)
            aggregate(hq_d, gin=False, cur_h=None, group_cb=fin_cb,
                      ag_issue=agi)

    lower_extended_insts(nc)
    _split_sync_waits(nc)
    return nc


def _make_weight_arrays(inp):
    f16 = np.float16
    return {
        'w0l': np.asarray(inp['sage0_wl'], np.float32).astype(f16),
        'w0r': np.asarray(inp['sage0_wr'], np.float32).astype(f16),
        'w1': np.asarray(inp['gin_w1'], np.float32).astype(f16),
        'w2': np.asarray(inp['gin_w2'], np.float32).astype(f16),
        'w2l': np.asarray(inp['sage2_wl'], np.float32).astype(f16),
        'w2r': np.asarray(inp['sage2_wr'], np.float32).astype(f16),
        'fwl': np.asarray(inp['fin_wl'], np.float32).astype(f16),
        'fwr': np.asarray(inp['fin_wr'], np.float32).astype(f16),
        'b1T': np.asarray(inp['gin_b1'], np.float32).reshape(P, 1),
        'finb': np.tile(np.asarray(inp['fin_b'], np.float32)[None, :],
                        (P, 1)),
        'bngT': np.ascontiguousarray(
            np.asarray(inp['bn_gamma'], np.float32).T),
        'bnbT': np.ascontiguousarray(
            np.asarray(inp['bn_beta'], np.float32).T),
        'iota': np.tile(np.arange(256, dtype=np.float32)[None, :],
                        (P, 1)).astype(f16),
        'ident': np.eye(P, dtype=np.float32).astype(f16),
    }


def _build_and_run(inputs, trace=False):
    _apply_tile_drain_patch()
    _install_ntff_hook()
    _maybe_reset_device()
    from concourse.bass_utils import run_bass_kernel_spmd

    x = np.asarray(inputs['x'], np.float32)
    ei = np.asarray(inputs['edge_index'])
    plan = build_plan2(x, ei)
    w = _make_weight_arrays(inputs)
    nc = build_bass2(plan)

    in_maps = []
    for c in range(NCORES):
        m = {
            'xT': plan.xT[c],
            'idxw': plan.idx_w[c],
            'ds128': plan.ds128[c],
            'ds256': plan.ds256[c],
            'cnt': plan.cnt[c],
            'invdeg': plan.invdeg[c],
        }
        for b in range(NB):
            m[f'xq{b}'] = plan.xq[b]
        m.update({k: w[k] for k in
                  ['iota', 'ident', 'w0l', 'w0r', 'w1', 'w2', 'w2l', 'w2r',
                   'fwl', 'fwr', 'b1T', 'finb', 'bngT', 'bnbT']})
        in_maps.append(m)
    res = run_bass_kernel_spmd(nc, in_maps, core_ids=list(range(NCORES)),
                               trace=trace)
    out = np.empty((x.shape[0], DOUT), np.float32)
    for c in range(NCORES):
        out[plan.orig_ids[c]] = res.results[c]['out'][:OWN, :DOUT]
    return out, res


def kernel(**inputs):
    out, _ = _build_and_run(inputs, trace=False)
    return out


def kernel_traced(**inputs):
    return _build_and_run(inputs, trace=True)


# revision 4
# speedup vs baseline: 1.0069x; 1.0069x over previous
"""Trainium2 Bass kernel v2 for nn_EnhancedGNN on 8 NeuronCores.

Key structure (vs v1):
- Nodes relabeled: in-degree-balanced assignment into 8 cores x 98 tiles so
  every tile sees ~2040 in-edges (tight max-over-core chunk counts).
- Node table stored as 4 quartile-block DRAM tensors hq[b] (rows =
  [core0 q-slice | core1 q-slice | ...], <=25600 rows so gather indices fit
  int16).  Layer-to-layer AllGather is split into 4 sub-AllGathers (one per
  quartile) issued right before the bucket that needs them -> next layer's
  gathers overlap the previous layer's apply/writeback/AllGather tail.
- One gather call per (source-quartile b, 512-col dst group g): ~17 chunks,
  ~2176 indices.  Per-core valid index count is loaded into a gpsimd
  register per call; padding index slots hold -1 and are SKIPPED by the
  SWDGE ucode (no DMA descriptors, no HBM traffic).
- Scatter one-hots stay 128 wide: each chunk gets a shared data-derived
  column window [o, o+w) (w=128, rarely 256) inside the 512-col group that
  covers every core's edge span for that chunk slot.
- PSUM group accumulator [128, 512] is zero-initialized by a matmul with an
  all-zero rhs, chunk matmuls accumulate into windows, one dummy stop
  matmul closes the bank.
- Writebacks (SBUF -> hown quartile, 256B rows) rotate across the sync /
  scalar / vector engine DMA queues.
"""
import dataclasses
import os
import sys
import types
import numpy as np

NOSKIP = bool(int(os.environ.get('K2_NOSKIP', '0')))
NOPIPE = bool(int(os.environ.get('K2_NOPIPE', '0')))
NOCC = bool(int(os.environ.get('K2_NOCC', '0')))
SPLIT1024 = bool(int(os.environ.get('K2_SPLIT1024', '0')))

# ---------------------------------------------------------------------------
# harness patches (walrus on this image encodes at most ONE sync wait per
# instruction; the axon NTFF profile hook is missing from the shipped antenv)
# ---------------------------------------------------------------------------


def _apply_tile_drain_patch():
    import concourse.tile as tile_mod
    from concourse.vector_clock import ScopedClock, VectorClock

    def _patched(self, tick_clock, wait_clock):
        nc = self.nc
        gc = tick_clock.global_clock
        n = len(gc)
        for i in range(n):
            t = gc[i]
            if t <= 0:
                continue
            vec = [0] * n
            vec[i] = t
            d = nc.sync.drain()
            wait_clock.add_sem_waits(d.ins, ScopedClock({None: VectorClock(vec)}))
        nc.sync.drain()
        nc.all_engine_barrier()
        assert self.sems is not None
        popped = nc._tile_sem_poison_stack.pop()
        assert popped is self._sem_poison
        nc.clear_and_free_semaphores(list(self.sems.allocated().values()))
        nc.all_engine_barrier()

    tile_mod.TileContext._drain_and_barrier = _patched


def _split_sync_waits(nc, max_waits=1):
    import concourse.mybir as mybir
    n_split = 0
    for f in nc.m.functions:
        for blk in f.blocks:
            new_insts = []
            for ins in blk.instructions:
                si = ins.sync_info
                if si is not None and si.on_wait and len(si.on_wait) > max_waits:
                    waits = list(si.on_wait)
                    keep = waits[-max_waits:]
                    for w in waits[:-max_waits]:
                        nop = mybir.InstNoOp(
                            name=f"{ins.name}-ws{n_split}", ins=[], outs=[])
                        nop.engine = ins.engine
                        nop.sync_info = mybir.SyncInfo(on_wait=[w], on_update=[])
                        new_insts.append(nop)
                        n_split += 1
                    si.on_wait = keep
                new_insts.append(ins)
            blk.instructions[:] = new_insts
    return n_split


def _install_ntff_hook():
    if 'antenv.axon_hooks' in sys.modules:
        return
    try:
        from trn_agent_boot.trn_boot import _ntff_profile_via_ctypes
        hook = _ntff_profile_via_ctypes('/opt/axon/libaxon_pjrt.so')
    except Exception:
        hook = None
    mod = types.ModuleType('antenv.axon_hooks')
    state = {'hook': hook}
    mod.get_axon_ntff_profile_hook = lambda: state['hook']
    mod.set_axon_ntff_profile_hook = lambda h: state.update(hook=h)
    sys.modules['antenv.axon_hooks'] = mod
    try:
        import antenv
        antenv.axon_hooks = mod
    except Exception:
        pass
    try:
        import concourse.bass_utils as bu
        bu.upload_artifacts = lambda tmpdir: tmpdir
    except Exception:
        pass


def _maybe_reset_device():
    try:
        import jax, ctypes
        jax.devices()
        lib = ctypes.CDLL('/opt/axon/libaxon_pjrt.so')
        lib.axon_reset.restype = ctypes.c_int64
        lib.axon_reset()
    except Exception:
        pass


# ---------------------------------------------------------------------------
# constants
# ---------------------------------------------------------------------------

H = 128
DOUT = 40
BN_EPS = 1e-5
NCORES = 8
P = 128
OWN = 12500
OWNP = 12544          # 98 tiles
T = 98
G = 25                # 512-col dst groups (last is 256 wide)
NB = 4                # source-quartile buckets
QS = [3200, 3200, 3200, 2944]       # quartile sizes (local rows)
QSTART = [0, 3200, 6400, 9600]
QT = [25, 25, 25, 23]               # tiles per quartile
QT0 = [0, 25, 50, 75, 98]           # tile ranges per quartile
GW = [512] * 24 + [256]
MIN_CNT = 16
GBUFS = 8      # gather/onehot pool buffers (in-flight calls)
AHEAD = 4      # call prefetch depth beyond current group's needs
IDXP = 16 if bool(int(os.environ.get('K2_IDX16', '0'))) else 128


@dataclasses.dataclass
class Plan2:
    ncalls: int
    gcalls: list    # per bucket: list of call dicts (<=8 slots each)
    groups: list    # per bucket: per g: [(call_k, pos, o, w, fam, fpos)]
    c128: int
    c256: int
    max_stw: int                    # max one-hot cols per call
    idx_len16: int                  # total idx stream length (16-lane units)
    bucket_idx16: list              # per bucket (start16, len16)
    idx_w: list                     # per core [128, idx_len16] int16
    ds128: list                     # per core [128, c128] f16
    ds256: list                     # per core [128, c256] f16
    cnt: list                       # per core [128, ncalls] int32
    invdeg: list                    # per core [128, OWNP] f16
    xT: list                        # per core [128, OWNP] f16
    xq: list                        # per bucket [8*QS[b], H] f16 (shared)
    orig_ids: list                  # per core [OWN] original node ids


def build_plan2(x, edge_index):
    N = x.shape[0]
    src = edge_index[0].astype(np.int64)
    dst = edge_index[1].astype(np.int64)

    # ---- degree-balanced node -> (core, local slot) assignment ----------
    deg = np.bincount(dst, minlength=N).astype(np.int64)
    order = np.argsort(-deg, kind='stable')
    nbins = NCORES * T
    cap = np.full(nbins, P, np.int64)
    cap[T - 1::T] = 84            # tile 97 of each core: 84 real + 44 pad
    import heapq
    heap = [(0, b) for b in range(nbins)]
    heapq.heapify(heap)
    fill = np.zeros(nbins, np.int64)
    core_of = np.empty(N, np.int32)
    loc_of = np.empty(N, np.int32)
    # bins assign slots in arrival order
    for v in order:
        while True:
            load, b = heapq.heappop(heap)
            if fill[b] < cap[b]:
                break
        c, t = divmod(b, T)
        core_of[v] = c
        loc_of[v] = t * P + fill[b]
        fill[b] += 1
        if fill[b] < cap[b]:
            heapq.heappush(heap, (load + deg[v], b))
    assert (fill == cap).all()

    orig_ids = []
    for c in range(NCORES):
        ids = np.full(OWN, -1, np.int64)
        m = core_of == c
        l = loc_of[m]
        # compress local slots (tile 97 has pads beyond 84 -> local ids of
        # real nodes are 0..12499 after shifting? no: slots are dense within
        # tiles; tile 97 fills 0..83 -> locals 12416..12499. dense already.)
        ids[l - (l // P - (T - 1)).clip(0) * 0] = np.where(m)[0]  # direct
        # simpler: direct placement
        ids = np.full(OWNP, -1, np.int64)
        ids[l] = np.where(m)[0]
        real = ids[ids >= 0]
        assert real.size == OWN
        # real locals are exactly 0..12499 by construction
        assert (ids[:OWN] >= 0).all() and (ids[OWN:] < 0).all()
        orig_ids.append(ids[:OWN])

    # ---- remap edges ----------------------------------------------------
    sc = core_of[src].astype(np.int64)
    sl = loc_of[src].astype(np.int64)
    dc = core_of[dst].astype(np.int64)
    dl = loc_of[dst].astype(np.int64)
    b_of = np.minimum(sl // 3200, 3)
    qs_arr = np.array(QS, np.int64)
    qstart_arr = np.array(QSTART, np.int64)
    idx16_of = sc * qs_arr[b_of] + (sl - qstart_arr[b_of])
    assert idx16_of.max() < 32768
    g_of = dl >> 9
    col_of = dl & 511

    # per-core sorted edge segments
    percore = []
    for c in range(NCORES):
        m = dc == c
        bb, gg, cc2, ii = b_of[m], g_of[m], col_of[m], idx16_of[m]
        o = np.lexsort((cc2, gg, bb))
        percore.append((bb[o], gg[o], cc2[o], ii[o]))

    # segment boundaries per (b, g) per core
    seg = np.zeros((NCORES, NB, G + 1), np.int64)
    for c in range(NCORES):
        bb, gg, _, _ = percore[c]
        key = bb * G + gg
        cnts = np.bincount(key, minlength=NB * G).reshape(NB, G)
        seg[c, :, 1:] = np.cumsum(cnts, axis=1)
        seg[c] += np.concatenate(
            [[0], np.cumsum(cnts.sum(axis=1))])[:-1, None]

    # ---- slot construction, then cross-group 8-slot call packing --------
    gcalls = [[] for _ in range(NB)]     # per bucket: list of call dicts
    groups = [[] for _ in range(NB)]     # per bucket: per g: slot refs
    idx_blocks = [[] for _ in range(NCORES)]
    ds128_cols = [[] for _ in range(NCORES)]
    ds256_cols = [[] for _ in range(NCORES)]
    cnt_rows = [[] for _ in range(NCORES)]
    bucket_idx16 = []
    idx_pos16 = 0
    max_stw = 0
    for b in range(NB):
        bstart16 = idx_pos16
        slot_recs = []                   # dicts: g, o, w, blocks, dscols, pad
        for g in range(G):
            gw = GW[g]
            e = [int(seg[c, b, g + 1] - seg[c, b, g]) for c in range(NCORES)]
            nsl = max(1, -(-max(e) // P))
            for j in range(nsl):
                lo, hi = 10 ** 9, -1
                for c in range(NCORES):
                    if e[c] <= j * P:
                        continue
                    s0 = int(seg[c, b, g])
                    cols = percore[c][2][s0 + j * P:s0 + min((j + 1) * P,
                                                             e[c])]
                    lo = min(lo, int(cols[0]))
                    hi = max(hi, int(cols[-1]))
                if hi < 0:
                    lo, hi = 0, 0
                span = hi - lo + 1
                w = 128 if span <= 128 else 256
                assert span <= 256, f"window span {span} at b{b} g{g} j{j}"
                w = min(w, gw)
                o = min(lo, gw - w)
                blocks, dscols, pad = [], [], 0
                for c in range(NCORES):
                    s0 = int(seg[c, b, g])
                    n = min(max(e[c] - j * P, 0), P)
                    blk = np.full(P, -1, np.int16)
                    dsv = np.full(P, -1.0, np.float32)
                    if n > 0:
                        blk[:n] = percore[c][3][s0 + j * P:
                                                s0 + j * P + n].astype(
                                                    np.int16)
                        dsv[:n] = percore[c][2][s0 + j * P:s0 + j * P + n] - o
                    blocks.append(blk)
                    dscols.append(dsv)
                    pad += P - n
                slot_recs.append(dict(g=g, o=o, w=w, blocks=blocks,
                                      dscols=dscols, pad=pad))
        # pack into calls of <=8 slots; within each window put the slot
        # with the most padding last so its pads become skippable
        for w0 in range(0, len(slot_recs), 8):
            win = slot_recs[w0:w0 + 8]
            last = max(range(len(win)), key=lambda i: win[i]['pad'])
            win = [win[i] for i in range(len(win)) if i != last] + [win[last]]
            nsl = len(win)
            nidx = nsl * P
            kloc = len(gcalls[b])
            n128 = sum(1 for s in win if s['w'] == 128)
            n256 = nsl - n128
            max_stw = max(max_stw, n128 * 128 + n256 * 256)
            # per-core idx assembly + counts
            for c in range(NCORES):
                lanes = []
                for pos, s in enumerate(win):
                    lane = s['blocks'][c].copy()
                    if pos < nsl - 1:
                        lane[lane < 0] = 0       # interior pads -> dummies
                    lanes.append(lane)
                blk = np.concatenate(lanes)
                evalid = int(np.max(np.nonzero(blk >= 0)[0],
                                    initial=-1)) + 1
                cnt_c = min(max(MIN_CNT, -(-evalid // 16) * 16), nidx)
                if NOSKIP:
                    cnt_c = nidx
                blk[evalid:cnt_c] = 0
                idx_blocks[c].append(blk)
                cnt_rows[c].append(cnt_c)
            # ds stream in call-slot order + group slot refs
            i128 = i256 = 0
            for pos, s in enumerate(win):
                if s['w'] == 128:
                    fam, fpos = 0, i128
                    i128 += 1
                    for c in range(NCORES):
                        ds128_cols[c].append(s['dscols'][c])
                else:
                    fam, fpos = 1, i256
                    i256 += 1
                    for c in range(NCORES):
                        ds256_cols[c].append(s['dscols'][c])
                while len(groups[b]) <= s['g']:
                    groups[b].append([])
                groups[b][s['g']].append((kloc, pos, s['o'], s['w'],
                                          fam, fpos))
            gcalls[b].append(dict(
                b=b, nslots=nsl, num_idxs=nidx, idx_off16=idx_pos16,
                cnt_idx=len(cnt_rows[0]) - 1, n128=n128, n256=n256,
                ds128_off=len(ds128_cols[0]) - n128,
                ds256_off=len(ds256_cols[0]) - n256))
            idx_pos16 += nidx // 16
        while len(groups[b]) < G:
            groups[b].append([])
        bucket_idx16.append((bstart16, idx_pos16 - bstart16))

    ncalls = sum(len(gc) for gc in gcalls)
    c128 = len(ds128_cols[0])
    c256 = max(1, len(ds256_cols[0]))

    idx_w, ds128_w, ds256_w, cnt_w = [], [], [], []
    for c in range(NCORES):
        flat = np.concatenate(idx_blocks[c])
        w16 = flat.reshape(-1, 16).T        # [16, idx_len16]
        idx_w.append(np.tile(w16, (8, 1)).astype(np.int16))
        d1 = np.stack(ds128_cols[c], axis=1).astype(np.float16)
        ds128_w.append(np.ascontiguousarray(d1))
        if ds256_cols[c]:
            d2 = np.stack(ds256_cols[c], axis=1).astype(np.float16)
        else:
            d2 = np.full((P, 1), -1.0, np.float16)
        ds256_w.append(np.ascontiguousarray(d2))
        cnt_w.append(np.tile(np.array(cnt_rows[c], np.int32)[None, :],
                             (P, 1)))

    # ---- invdeg / x staging --------------------------------------------
    inv_all = (1.0 / np.maximum(deg, 1.0)).astype(np.float32)
    invs, xTs = [], []
    xf = np.asarray(x, np.float32)
    for c in range(NCORES):
        iv = np.zeros(OWNP, np.float32)
        iv[:OWN] = inv_all[orig_ids[c]]
        invs.append(np.tile(iv[None, :], (P, 1)).astype(np.float16))
        xo = np.zeros((OWNP, H), np.float32)
        xo[:OWN] = xf[orig_ids[c]]
        xTs.append(np.ascontiguousarray(xo.T).astype(np.float16))

    xq = []
    for b in range(NB):
        tb = np.zeros((NCORES * QS[b], H), np.float16)
        for c in range(NCORES):
            q0, q1 = QSTART[b], QSTART[b] + QS[b]
            rows = np.zeros((QS[b], H), np.float32)
            hi = min(q1, OWN)
            if hi > q0:
                rows[:hi - q0] = xf[orig_ids[c][q0:hi]]
            tb[c * QS[b]:(c + 1) * QS[b]] = rows.astype(np.float16)
        xq.append(tb)

    return Plan2(ncalls=ncalls, gcalls=gcalls, groups=groups,
                 c128=c128, c256=c256,
                 max_stw=max_stw, idx_len16=idx_pos16,
                 bucket_idx16=bucket_idx16, idx_w=idx_w, ds128=ds128_w,
                 ds256=ds256_w, cnt=cnt_w, invdeg=invs, xT=xTs, xq=xq,
                 orig_ids=orig_ids)


def _bcast_mid(ap, reps):
    return dataclasses.replace(ap, ap=[ap.ap[0], [0, reps], ap.ap[1]])


def _bcast_last(ap, reps):
    return dataclasses.replace(ap, ap=[ap.ap[0], ap.ap[1], [0, reps]])


def build_bass2(pl):
    import concourse.bass as bass
    import concourse.mybir as mybir
    from concourse.tile import TileContext
    from concourse import library_config
    from concourse.library_overlay import lower_extended_insts

    F16, F32, I16, I32 = (mybir.dt.float16, mybir.dt.float32,
                          mybir.dt.int16, mybir.dt.int32)
    AX = mybir.AxisListType
    ALU = mybir.AluOpType
    ACTF = mybir.ActivationFunctionType

    nc = bass.Bass('TRN2', target_bir_lowering=False, debug=False,
                   num_devices=NCORES, num_swdge_queues=4)

    def din(name, shape, dt):
        return nc.dram_tensor(name, shape, dt, kind='ExternalInput')

    xq_d = [din(f'xq{b}', [NCORES * QS[b], H], F16) for b in range(NB)]
    xT_d = din('xT', [P, OWNP], F16)
    idx_d = din('idxw', [P, pl.idx_len16], I16)
    ds128_d = din('ds128', [P, pl.c128], F16)
    ds256_d = din('ds256', [P, pl.c256], F16)
    cnt_d = din('cnt', [P, pl.ncalls], I32)
    inv_d = din('invdeg', [P, OWNP], F16)
    iota_d = din('iota', [P, 256], F16)
    ident_d = din('ident', [P, P], F16)
    wname = ['w0l', 'w0r', 'w1', 'w2', 'w2l', 'w2r']
    w_d = {k: din(k, [H, H], F16) for k in wname}
    fwl_d = din('fwl', [H, DOUT], F16)
    fwr_d = din('fwr', [H, DOUT], F16)
    b1_d = din('b1T', [P, 1], F32)
    finb_d = din('finb', [P, DOUT], F32)
    bng_d = din('bngT', [P, 3], F32)
    bnb_d = din('bnbT', [P, 3], F32)
    OUTP = 64
    out_d = nc.dram_tensor('out', [OWNP, OUTP], F32, kind='ExternalOutput')

    hq_own_d = [nc.dram_tensor(f'hqo{b}', [QS[b], H], F16, kind='Internal')
                for b in range(NB)]
    hq_shared = not bool(int(os.environ.get('K2_NOSHARED', '0')))
    hq_d = [nc.dram_tensor(f'hq{b}', [NCORES * QS[b], H], F16,
                           kind='Internal',
                           addr_space='Shared' if hq_shared else 'Local')
            for b in range(NB)]
    stin_d = [nc.dram_tensor(f'stin{i}', [P, 2], F32, kind='Internal')
              for i in range(3)]
    stout_d = [nc.dram_tensor(f'stout{i}', [P, 2], F32, kind='Internal',
                              addr_space='Shared') for i in range(3)]
    rg = [list(range(NCORES))]

    nc.gpsimd.load_library(library_config.mlp)

    NGRP = G
    grp_w = [GW[g] for g in range(G)]
    grp_v = [min(512, max(0, OWN - g * 512)) for g in range(G)]

    with TileContext(nc) as tc:
        cnt_reg = nc.alloc_register(mybir.EngineType.Pool, 'gcnt')
        nidx_regs = {n: nc.gpsimd.to_reg(n)
                     for n in sorted({cl['num_idxs'] for gc in pl.gcalls
                                      for cl in gc})}
        import contextlib
        ctx = contextlib.ExitStack()
        with ctx:
            persist = ctx.enter_context(tc.tile_pool(name='persist', bufs=1))
            gpool = ctx.enter_context(tc.tile_pool(name='g', bufs=GBUFS))
            spool = ctx.enter_context(tc.tile_pool(name='s', bufs=GBUFS))
            ipool = ctx.enter_context(tc.tile_pool(name='idx', bufs=2))
            epool = ctx.enter_context(tc.tile_pool(name='evac', bufs=3))
            psA = ctx.enter_context(
                tc.tile_pool(name='psA', bufs=3, space='PSUM'))
            psD = ctx.enter_context(
                tc.tile_pool(name='psD', bufs=2, space='PSUM'))
            psT = ctx.enter_context(
                tc.tile_pool(name='psT', bufs=2, space='PSUM'))

            def load(name, shape, dt, srcap):
                t = persist.tile(shape, dt, tag=name)
                nc.sync.dma_start(t[:], srcap)
                return t

            ds128_sb = load('ds128', [P, pl.c128], F16, ds128_d[:])
            ds256_sb = load('ds256', [P, pl.c256], F16, ds256_d[:])
            cnt_sb = load('cnt', [P, pl.ncalls], I32, cnt_d[:])
            inv_sb = load('inv', [P, OWNP], F16, inv_d[:])
            iota_sb = load('iota', [P, 256], F16, iota_d[:])
            ident_sb = load('ident', [P, P], F16, ident_d[:])
            w_sb = {k: load(k, [H, H], F16, w_d[k][:]) for k in wname}
            fwl_sb = load('fwl', [H, DOUT], F16, fwl_d[:])
            fwr_sb = load('fwr', [H, DOUT], F16, fwr_d[:])
            b1_sb = load('b1', [P, 1], F32, b1_d[:])
            finb_sb = load('finb', [P, DOUT], F32, finb_d[:])
            bng_sb = load('bng', [P, 3], F32, bng_d[:])
            bnb_sb = load('bnb', [P, 3], F32, bnb_d[:])
            xqsb = load('xq', [P, OWNP], F16, xT_d[:])

            hA = persist.tile([P, OWNP], F16, tag='hA')
            hB = persist.tile([P, OWNP], F16, tag='hB')
            aggT = persist.tile([P, OWNP], F16, tag='aggT')
            zero512 = persist.tile([P, 512], F16, tag='zero512')
            sums = persist.tile([P, NGRP], F32, tag='sums')
            sqs = persist.tile([P, NGRP], F32, tag='sqs')
            stat = persist.tile([P, 2], F32, tag='stat')
            gstat = persist.tile([P, 2], F32, tag='gstat')
            scl = persist.tile([P, 1], F32, tag='scl')
            bia = persist.tile([P, 1], F32, tag='bia')
            tmp1 = persist.tile([P, 1], F32, tag='tmp1')
            tmp2 = persist.tile([P, 1], F32, tag='tmp2')
            ttscr = persist.tile([P, 512], F32, tag='ttscr')
            logit = persist.tile([P, T * DOUT], F32, tag='logit')
            mx = persist.tile([P, T], F32, tag='mx')
            lse = persist.tile([P, T], F32, tag='lse')
            escr = persist.tile([P, DOUT], F32, tag='escr')

            nc.vector.memset(zero512[:], 0.0)

            # zero the gather-pool buffers once (pad slots are skipped by
            # SWDGE and must never expose NaN bit patterns to the matmul)
            maxsl = 8
            for _ in range(GBUFS):
                t = gpool.tile([P, maxsl, P], F16, tag='g')
                nc.vector.memset(
                    t[:].rearrange('p a b -> p (a b)'), 0.0)

            # collective warm-up — emitted a few gather calls into layer 0
            # so the first gathers start immediately at kernel entry
            def warm_cb():
                for i in range(2):
                    nc.gpsimd.collective_compute(
                        'AllReduce', ALU.add, rg, ins=[stin_d[i][:]],
                        outs=[stout_d[i][:]])
                if not NOCC:
                    nc.gpsimd.collective_compute(
                        'AllGather', ALU.bypass, rg,
                        ins=[hq_own_d[0][0:2, :]],
                        outs=[hq_d[0][0:2 * NCORES, :]])

            # ---------------- aggregation -----------------------------
            def aggregate(tables, gin, cur_h, group_cb, ag_issue=None,
                          mid_cb=None):
                for b in range(NB):
                    if ag_issue is not None:
                        ag_issue[b]()
                    s16, l16 = pl.bucket_idx16[b]
                    h1 = (l16 // 2 + 7) // 8 * 8
                    it = ipool.tile([P, l16], I16, tag='idx')
                    nc.scalar.dma_start(it[:, :h1], idx_d[:, s16:s16 + h1])
                    nc.scalar.dma_start(it[:, h1:l16],
                                        idx_d[:, s16 + h1:s16 + l16])
                    bcalls = pl.gcalls[b]
                    # last group consuming each call (for pend release)
                    last_grp = {}
                    for g in range(G):
                        for (kk, *_rest) in pl.groups[b][g]:
                            last_grp[kk] = g
                    pend = {}
                    emitted = [0]

                    def emit_upto(klim, itile, s16=s16, bcalls=bcalls,
                                  pend=pend, emitted=emitted):
                        while emitted[0] < min(klim, len(bcalls)):
                            k = emitted[0]
                            cl = bcalls[k]
                            nsl = cl['nslots']
                            sni = cl['num_idxs']
                            if NOSKIP:
                                nreg = nidx_regs[sni]
                            else:
                                nc.gpsimd.reg_load(
                                    cnt_reg,
                                    cnt_sb[0:1, cl['cnt_idx']:
                                           cl['cnt_idx'] + 1])
                                nreg = cnt_reg
                            gt = gpool.tile([P, maxsl, P], F16, tag='g')
                            nc.gpsimd.dma_gather(
                                gt[:, :nsl, :], tables[b][:],
                                itile[0:IDXP,
                                      cl['idx_off16'] - s16:
                                      cl['idx_off16'] - s16 + sni // 16],
                                num_idxs=sni, num_idxs_reg=nreg,
                                elem_size=H, queue_num=k % 4)
                            n128, n256 = cl['n128'], cl['n256']
                            st = spool.tile([P, pl.max_stw], F16, tag='s')
                            if n128:
                                o = cl['ds128_off']
                                nc.vector.tensor_tensor(
                                    out=st[:, :n128 * 128].rearrange(
                                        'p (c f) -> p c f', c=n128),
                                    in0=_bcast_mid(iota_sb[:, :128], n128),
                                    in1=_bcast_last(
                                        ds128_sb[:, o:o + n128], 128),
                                    op=ALU.is_equal)
                            if n256:
                                o = cl['ds256_off']
                                nc.vector.tensor_tensor(
                                    out=st[:, n128 * 128:
                                           n128 * 128 + n256 * 256]
                                    .rearrange('p (c f) -> p c f', c=n256),
                                    in0=_bcast_mid(iota_sb[:, :256], n256),
                                    in1=_bcast_last(
                                        ds256_sb[:, o:o + n256], 256),
                                    op=ALU.is_equal)
                            pend[k] = (gt, st, n128)
                            emitted[0] += 1

                    for g in range(G):
                        if mid_cb is not None and b == 0 and g == 2:
                            mid_cb()
                        gw = GW[g]
                        slots = pl.groups[b][g]
                        kmax = max((kk for (kk, *_r) in slots), default=-1)
                        emit_upto(kmax + 1 + AHEAD, it)
                        pt = psA.tile([P, 512], F32, tag='agg')
                        nc.tensor.matmul(
                            pt[:, :gw], lhsT=ident_sb[:],
                            rhs=zero512[:, :gw], start=True, stop=False,
                            skip_group_check=True)
                        if gin and b == 0:
                            for tt in range(gw // 128):
                                t0 = (g * 4 + tt) * P
                                nc.tensor.matmul(
                                    pt[:, tt * 128:tt * 128 + 128],
                                    lhsT=ident_sb[:],
                                    rhs=cur_h[:, t0:t0 + P],
                                    start=False, stop=False,
                                    skip_group_check=True)
                        for (kk, pos, o, w, fam, fpos) in slots:
                            gt, st, n128 = pend[kk]
                            if fam == 0:
                                rhs = st[:, fpos * 128:(fpos + 1) * 128]
                            else:
                                rhs = st[:, n128 * 128 + fpos * 256:
                                         n128 * 128 + (fpos + 1) * 256]
                            nc.tensor.matmul(
                                pt[:, o:o + w], lhsT=gt[:, pos, :], rhs=rhs,
                                start=False, stop=False,
                                skip_group_check=True)
                        nc.tensor.matmul(
                            pt[:, :gw], lhsT=ident_sb[:],
                            rhs=zero512[:, :gw], start=False, stop=True,
                            skip_group_check=True)
                        for kk in [kk for kk, lg in last_grp.items()
                                   if lg == g]:
                            pend.pop(kk, None)
                        sl = aggT[:, g * 512:g * 512 + gw]
                        if b == 0:
                            nc.vector.tensor_copy(out=sl, in_=pt[:, :gw])
                        else:
                            nc.vector.tensor_tensor(
                                out=sl, in0=sl, in1=pt[:, :gw], op=ALU.add)
                        if b == NB - 1:
                            if not gin:
                                nc.vector.tensor_tensor(
                                    out=sl, in0=sl,
                                    in1=inv_sb[:, g * 512:g * 512 + gw],
                                    op=ALU.mult)
                            group_cb(g)

            # ---------------- dense / stats ---------------------------
            def dense_prebn(layer, g, cur):
                w = grp_w[g]
                pt = psD.tile([P, 512], F32, tag='dense')
                sl = slice(g * 512, g * 512 + w)
                if layer == 0:
                    nc.tensor.matmul(pt[:, :w], lhsT=w_sb['w0l'][:],
                                     rhs=aggT[:, sl], start=True, stop=False,
                                     skip_group_check=True)
                    nc.tensor.matmul(pt[:, :w], lhsT=w_sb['w0r'][:],
                                     rhs=xqsb[:, sl], start=False, stop=True,
                                     skip_group_check=True)
                elif layer == 1:
                    nc.tensor.matmul(pt[:, :w], lhsT=w_sb['w2'][:],
                                     rhs=xqsb[:, sl], start=True, stop=True,
                                     skip_group_check=True)
                else:
                    nc.tensor.matmul(pt[:, :w], lhsT=w_sb['w2l'][:],
                                     rhs=aggT[:, sl], start=True, stop=False,
                                     skip_group_check=True)
                    nc.tensor.matmul(pt[:, :w], lhsT=w_sb['w2r'][:],
                                     rhs=cur[:, sl], start=False, stop=True,
                                     skip_group_check=True)
                return pt

            def stats_pass1(layer, cur, scratch):
                def cb(g):
                    w, v = grp_w[g], grp_v[g]
                    pt = dense_prebn(layer, g, cur)
                    sl = slice(g * 512, g * 512 + w)
                    nc.scalar.copy(scratch[:, sl], pt[:, :w])
                    if v > 0:
                        nc.vector.tensor_reduce(
                            out=sums[:, g:g + 1], in_=pt[:, :v], axis=AX.X,
                            op=ALU.add)
                        nc.scalar.activation(
                            ttscr[:, :v], pt[:, :v], ACTF.Square,
                            accum_out=sqs[:, g:g + 1])
                    else:
                        nc.vector.memset(sums[:, g:g + 1], 0.0)
                        nc.vector.memset(sqs[:, g:g + 1], 0.0)
                return cb

            WBT = 4
            wb_engines = [nc.sync, nc.scalar]
            wb_rr = [0]

            def bn_stats_and_apply(layer, cur, nxt, residual, scratch):
                nc.vector.tensor_reduce(out=stat[:, 0:1], in_=sums[:],
                                        axis=AX.X, op=ALU.add)
                nc.vector.tensor_reduce(out=stat[:, 1:2], in_=sqs[:],
                                        axis=AX.X, op=ALU.add)
                nc.sync.dma_start(stin_d[layer][:], stat[:])
                nc.gpsimd.collective_compute(
                    'AllReduce', ALU.add, rg, ins=[stin_d[layer][:]],
                    outs=[stout_d[layer][:]])
                nc.sync.dma_start(gstat[:], stout_d[layer][:])
                invN = 1.0 / 100000.0
                nc.vector.tensor_scalar_mul(tmp1[:], gstat[:, 0:1], invN)
                nc.vector.tensor_scalar_mul(tmp2[:], gstat[:, 1:2], invN)
                nc.vector.tensor_tensor(out=scl[:], in0=tmp1[:], in1=tmp1[:],
                                        op=ALU.mult)
                nc.vector.tensor_tensor(out=tmp2[:], in0=tmp2[:], in1=scl[:],
                                        op=ALU.subtract)
                nc.vector.tensor_scalar_add(tmp2[:], tmp2[:], BN_EPS)
                nc.scalar.sqrt(tmp2[:], tmp2[:])
                nc.vector.reciprocal(tmp2[:], tmp2[:])
                nc.vector.tensor_tensor(out=scl[:],
                                        in0=bng_sb[:, layer:layer + 1],
                                        in1=tmp2[:], op=ALU.mult)
                nc.vector.tensor_tensor(out=tmp1[:], in0=tmp1[:], in1=scl[:],
                                        op=ALU.mult)
                nc.vector.tensor_tensor(out=bia[:],
                                        in0=bnb_sb[:, layer:layer + 1],
                                        in1=tmp1[:], op=ALU.subtract)

                def wb_tiles(q, t0, nt):
                    pt2 = psT.tile([P, WBT * P], F16, tag='trf16')
                    for j in range(nt):
                        t = t0 + j
                        nc.tensor.transpose(
                            pt2[:, j * P:(j + 1) * P],
                            nxt[:, t * P:(t + 1) * P], ident_sb[:])
                    et = epool.tile([P, WBT * P], F16, tag='ev')
                    nc.vector.tensor_copy(out=et[:, :nt * P],
                                          in_=pt2[:, :nt * P])
                    r0 = (t0 - QT0[q]) * P
                    dst = hq_own_d[q][r0:r0 + nt * P, :]
                    dst = dataclasses.replace(
                        dst, ap=[[H, P], [P * H, nt], [1, H]])
                    eng = wb_engines[wb_rr[0] % len(wb_engines)]
                    wb_rr[0] += 1
                    eng.dma_start(
                        dst, et[:, :nt * P].rearrange(
                            'p (j f) -> p j f', j=nt))

                # apply groups in order; writeback + AG per quartile
                qnext = [0]
                t_done = [0]

                def flush_quartile_upto(tile_lim):
                    while qnext[0] < NB and QT0[qnext[0] + 1] <= tile_lim:
                        q = qnext[0]
                        while t_done[0] < QT0[q + 1]:
                            nt = min(WBT, QT0[q + 1] - t_done[0])
                            wb_tiles(q, t_done[0], nt)
                            t_done[0] += nt
                        qnext[0] += 1

                for g in range(NGRP):
                    w = grp_w[g]
                    sl = slice(g * 512, g * 512 + w)
                    nc.scalar.activation(nxt[:, sl], scratch[:, sl],
                                         ACTF.Relu, bias=bia[:], scale=scl[:])
                    if residual:
                        nc.vector.tensor_tensor(out=nxt[:, sl],
                                                in0=nxt[:, sl],
                                                in1=cur[:, sl], op=ALU.add)
                    if g == NGRP - 1:
                        nc.vector.memset(nxt[:, OWN:OWNP], 0.0)
                        flush_quartile_upto(T)
                    else:
                        flush_quartile_upto(((g + 1) * 512) // P)

                def make_issue(q):
                    def issue():
                        if NOCC:
                            nc.sync.dma_start(hq_d[q][0:QS[q], :],
                                              hq_own_d[q][:])
                            return
                        nc.gpsimd.collective_compute(
                            'AllGather', ALU.bypass, rg,
                            ins=[hq_own_d[q][:]], outs=[hq_d[q][:]])
                    return issue
                if NOPIPE:
                    for q in range(NB):
                        make_issue(q)()
                    return [(lambda: None) for _ in range(NB)]
                return [make_issue(q) for q in range(NB)]

            # ---------------- final layer softmax ---------------------
            SM_G = [5, 11, 15, 18, 20, 22, 23, NGRP - 1]
            sm_done = [0]

            def softmax_flush(te):
                ts = sm_done[0]
                nt = te - ts
                if nt <= 0:
                    return
                lv = logit[:, ts * DOUT:te * DOUT].rearrange(
                    'p (t c) -> p t c', t=nt)
                nc.vector.tensor_reduce(out=mx[:, ts:te], in_=lv, axis=AX.X,
                                        op=ALU.max)
                nc.vector.tensor_tensor(
                    out=lv, in0=lv, in1=_bcast_last(mx[:, ts:te], DOUT),
                    op=ALU.subtract)
                for t in range(ts, te):
                    nc.scalar.activation(
                        escr[:], logit[:, t * DOUT:(t + 1) * DOUT], ACTF.Exp,
                        accum_out=lse[:, t:t + 1])
                nc.scalar.activation(lse[:, ts:te], lse[:, ts:te], ACTF.Ln)
                nc.vector.tensor_tensor(
                    out=lv, in0=lv, in1=_bcast_last(lse[:, ts:te], DOUT),
                    op=ALU.subtract)
                dstap = out_d[ts * P:te * P, :DOUT]
                dstap = dataclasses.replace(
                    dstap, ap=[[OUTP, P], [P * OUTP, nt], [1, DOUT]])
                nc.sync.dma_start(dstap, lv)
                sm_done[0] = te

            def fin_cb(g):
                n_t = min(g * 4 + 4, T) - g * 4
                for k in range(n_t):
                    t = g * 4 + k
                    pt = psD.tile([P, DOUT], F32, tag='dense')
                    nc.tensor.matmul(pt[:, :DOUT],
                                     lhsT=aggT[:, t * P:(t + 1) * P],
                                     rhs=fwl_sb[:], start=True, stop=False,
                                     skip_group_check=True)
                    nc.tensor.matmul(pt[:, :DOUT],
                                     lhsT=hA[:, t * P:(t + 1) * P],
                                     rhs=fwr_sb[:], start=False, stop=True,
                                     skip_group_check=True)
                    nc.vector.tensor_tensor(
                        out=logit[:, t * DOUT:(t + 1) * DOUT],
                        in0=pt[:, :DOUT], in1=finb_sb[:], op=ALU.add)
                if g in SM_G:
                    softmax_flush(min(g * 4 + 4, T))

            # ---------------- layer sequence --------------------------
            warm_cb()
            aggregate(xq_d, gin=False, cur_h=None,
                      group_cb=stats_pass1(0, None, hB))
            agi = bn_stats_and_apply(0, cur=None, nxt=hA, residual=False,
                                     scratch=hB)
            _p1_gin = stats_pass1(1, hA, aggT)

            def gin_cb(g):
                w = grp_w[g]
                pt = psD.tile([P, 512], F32, tag='dense')
                sl = slice(g * 512, g * 512 + w)
                nc.tensor.matmul(pt[:, :w], lhsT=w_sb['w1'][:],
                                 rhs=aggT[:, sl], start=True, stop=True,
                                 skip_group_check=True)
                nc.scalar.activation(xqsb[:, sl], pt[:, :w], ACTF.Relu,
                                     bias=b1_sb[:], scale=1.0)
                _p1_gin(g)

            aggregate(hq_d, gin=True, cur_h=hA, group_cb=gin_cb,
                      ag_issue=agi)
            agi = bn_stats_and_apply(1, cur=hA, nxt=hB, residual=True,
                                     scratch=aggT)
            aggregate(hq_d, gin=False, cur_h=None,
                      group_cb=stats_pass1(2, hB, xqsb), ag_issue=agi)
            agi = bn_stats_and_apply(2, cur=hB, nxt=hA, residual=True,
                                     scratch=xqsb)
            aggregate(hq_d, gin=False, cur_h=None, group_cb=fin_cb,
                      ag_issue=agi)

    lower_extended_insts(nc)
    _split_sync_waits(nc)
    return nc


def _make_weight_arrays(inp):
    f16 = np.float16
    return {
        'w0l': np.asarray(inp['sage0_wl'], np.float32).astype(f16),
        'w0r': np.asarray(inp['sage0_wr'], np.float32).astype(f16),
        'w1': np.asarray(inp['gin_w1'], np.float32).astype(f16),
        'w2': np.asarray(inp['gin_w2'], np.float32).astype(f16),
        'w2l': np.asarray(inp['sage2_wl'], np.float32).astype(f16),
        'w2r': np.asarray(inp['sage2_wr'], np.float32).astype(f16),
        'fwl': np.asarray(inp['fin_wl'], np.float32).astype(f16),
        'fwr': np.asarray(inp['fin_wr'], np.float32).astype(f16),
        'b1T': np.asarray(inp['gin_b1'], np.float32).reshape(P, 1),
        'finb': np.tile(np.asarray(inp['fin_b'], np.float32)[None, :],
                        (P, 1)),
        'bngT': np.ascontiguousarray(
            np.asarray(inp['bn_gamma'], np.float32).T),
        'bnbT': np.ascontiguousarray(
            np.asarray(inp['bn_beta'], np.float32).T),
        'iota': np.tile(np.arange(256, dtype=np.float32)[None, :],
                        (P, 1)).astype(f16),
        'ident': np.eye(P, dtype=np.float32).astype(f16),
    }


def _build_and_run(inputs, trace=False):
    _apply_tile_drain_patch()
    _install_ntff_hook()
    _maybe_reset_device()
    from concourse.bass_utils import run_bass_kernel_spmd

    x = np.asarray(inputs['x'], np.float32)
    ei = np.asarray(inputs['edge_index'])
    plan = build_plan2(x, ei)
    w = _make_weight_arrays(inputs)
    nc = build_bass2(plan)

    in_maps = []
    for c in range(NCORES):
        m = {
            'xT': plan.xT[c],
            'idxw': plan.idx_w[c],
            'ds128': plan.ds128[c],
            'ds256': plan.ds256[c],
            'cnt': plan.cnt[c],
            'invdeg': plan.invdeg[c],
        }
        for b in range(NB):
            m[f'xq{b}'] = plan.xq[b]
        m.update({k: w[k] for k in
                  ['iota', 'ident', 'w0l', 'w0r', 'w1', 'w2', 'w2l', 'w2r',
                   'fwl', 'fwr', 'b1T', 'finb', 'bngT', 'bnbT']})
        in_maps.append(m)
    res = run_bass_kernel_spmd(nc, in_maps, core_ids=list(range(NCORES)),
                               trace=trace)
    out = np.empty((x.shape[0], DOUT), np.float32)
    for c in range(NCORES):
        out[plan.orig_ids[c]] = res.results[c]['out'][:OWN, :DOUT]
    return out, res


def kernel(**inputs):
    out, _ = _build_and_run(inputs, trace=False)
    return out


def kernel_traced(**inputs):
    return _build_and_run(inputs, trace=True)
